# revision 32
# baseline (speedup 1.0000x reference)
"""Causal multi-head self-attention on 8 trn2 NeuronCores.

Sharding: 8 cores = 4 batch x 2 head-groups. Core i handles batch i//2 and
heads (i%2)*8 .. (i%2)*8+8 (8 of 16 heads, 512 of 1024 d_model columns).
Each core computes a full (2048, 1024) partial output (its head group pushed
through its w_proj row-slice); the host sums the two partials per batch
element (the tensor-parallel all-reduce done host-side).

Per-core dataflow (everything in transposed layout to avoid transposing the
big attention intermediates):
  x^T        : PE-transpose of x tiles (128 transposes of [128,128])
  Q^T, K^T   : w_q/w_k stationary, x^T moving  -> [cols, seq] layout
  V          : x^T stationary, w_v moving      -> natural [seq, cols] layout,
               stored with a ones-column per head (V_aug [128, 65]) so the
               softmax denominator rides along the AV matmul as output row 64
  S^T        : K^T stationary, Q^T moving, two heads packed into the 128 PE
               rows (contraction = head_dim 64, partitions 0-63 / 64-127)
  P^T        : exp(S^T * rsqrt(hd)) on ScalarE (no max-subtraction needed:
               |S|*rsqrt stays < ~10), causal handled by skipping k>q chunks,
               zeroing the invalid prefix, and a 0/1 upper-tri mask multiply
               on the diagonal 128x128 block
  O^T_aug    : V_aug stationary, P^T moving, accumulated over k-tiles in PSUM
  O^T        : O^T_aug rows 0-63 * reciprocal(row 64) (partition-broadcast)
  out        : O^T stationary, w_proj rows moving -> natural [seq, 1024]
All matmuls use float32r (full-rate fp32 on the PE for moving dim >= 256).
"""

import numpy as np

import concourse.bass as bass
import concourse.mybir as mybir
import concourse.tile as tile
from concourse import bacc
from concourse.bass_utils import run_bass_kernel_spmd
from concourse.masks import make_identity, make_upper_triangular

F32 = mybir.dt.float32
F32R = mybir.dt.float32r
BF16 = mybir.dt.bfloat16
AF = mybir.ActivationFunctionType

SEQ = 2048
DM = 1024
COLS = 512          # head-cols per core (8 heads x 64)
HD = 64
P = 128
N_CORES = 8
RSQRT = 0.125       # 1/sqrt(64)

SEQ_T = SEQ // P    # 16 seq tiles
DM_T = DM // P      # 8 d_model tiles
QC = 512            # q-chunk (PSUM free size)
N_QC = SEQ // QC    # 4 q chunks
KT_PER_QC = QC // P  # 4 k-tiles per q chunk


def _build_core_program():
    nc = bacc.Bacc(
        "TRN2", target_bir_lowering=False, debug=False, num_devices=N_CORES
    )
    x = nc.dram_tensor("x", [SEQ, DM], F32, kind="ExternalInput").ap()
    wq = nc.dram_tensor("wq", [DM, COLS], F32, kind="ExternalInput").ap()
    wk = nc.dram_tensor("wk", [DM, COLS], F32, kind="ExternalInput").ap()
    wv = nc.dram_tensor("wv", [DM, COLS], F32, kind="ExternalInput").ap()
    wp = nc.dram_tensor("wp", [COLS, DM], F32, kind="ExternalInput").ap()
    out = nc.dram_tensor("out", [SEQ, DM], F32, kind="ExternalOutput").ap()

    with tile.TileContext(nc) as tc:
        _emit(tc, x, wq, wk, wv, wp, out)
    nc.compile()
    return nc


def _emit(tc, x, wq, wk, wv, wp, out):
    nc = tc.nc

    # --- program-lifetime pools -------------------------------------------
    const_pool = tc.alloc_tile_pool(name="const", bufs=1)
    psum_mm = tc.alloc_tile_pool(name="psum_mm", bufs=4, space="PSUM")
    psum_acc = tc.alloc_tile_pool(name="psum_acc", bufs=4, space="PSUM")

    ident = const_pool.tile([P, P], F32, tag="ident")
    make_identity(nc, ident[:])
    mask01 = const_pool.tile([P, P], BF16, tag="mask01")
    # 1.0 where free-idx (q) >= partition-idx (k), else 0 — causal in S^T layout
    make_upper_triangular(nc, mask01[:], val=1.0, diag=True)
    # fp32r constants must be produced by a rounding engine write, not memset
    cstage = const_pool.tile([P, 3 * P], F32, tag="cstage")
    nc.vector.memset(cstage[:], 0.0)
    zeros384 = const_pool.tile([P, 3 * P], BF16, tag="zeros384")
    nc.vector.tensor_copy(zeros384[:], cstage[:])
    nc.vector.memset(cstage[:, 0:HD], 1.0)

    # --- long-lived intermediates -----------------------------------------
    xt_pool = tc.alloc_tile_pool(name="xt", bufs=1)
    xt = xt_pool.tile([P, DM_T * SEQ], F32R, tag="xt")  # [128, 8*2048], x^T

    # Q^T/K^T bounce through DRAM (SBUF can't hold them + everything else).
    # A DRAM tile pool keeps Tile's dependency tracking across the round trip.
    qk_dram_pool = tc.alloc_tile_pool(name="qk_dram", bufs=1, space="DRAM")
    qk_dram = qk_dram_pool.tile([P, 2 * 4 * SEQ], BF16, tag="qkdram")
    qk_dram_v = qk_dram[:].rearrange("p (w c s) -> p w c s", w=2, c=4, s=SEQ)

    # ===== phase A: load x, build x^T =====================================
    xload_pool = tc.alloc_tile_pool(name="xload", bufs=3)
    for s in range(SEQ_T):
        xin = xload_pool.tile([P, DM], F32, tag="xin")
        nc.sync.dma_start(xin[:], x[s * P : (s + 1) * P, :])
        for d in range(DM_T):
            pt = psum_mm.tile([P, P], F32, tag="mm")
            nc.tensor.transpose(pt[:], xin[:, d * P : (d + 1) * P], ident[:])
            nc.vector.tensor_copy(
                xt[:, d * SEQ + s * P : d * SEQ + (s + 1) * P], pt[:]
            )
    xload_pool.release()

    # ===== phase B: Q^T, K^T ==============================================
    wqk_pool = tc.alloc_tile_pool(name="wqk", bufs=1)
    wq_sb = wqk_pool.tile([P, DM_T * COLS], F32R, tag="wq_sb")
    wk_sb = wqk_pool.tile([P, DM_T * COLS], F32R, tag="wk_sb")
    wstg_pool = tc.alloc_tile_pool(name="wstg", bufs=3)
    for w_dram, w_sb in ((wq, wq_sb), (wk, wk_sb)):
        for d in range(DM_T):
            wst = wstg_pool.tile([P, COLS], F32, tag="wst")
            nc.sync.dma_start(wst[:], w_dram[d * P : (d + 1) * P, :])
            nc.vector.tensor_copy(
                w_sb[:, d * COLS : (d + 1) * COLS], wst[:]
            )
    wstg_pool.release()
    qkstg_pool = tc.alloc_tile_pool(name="qkstg", bufs=3)
    for w_i, w_sb in ((0, wq_sb), (1, wk_sb)):
        for c in range(4):  # head-pair col tile
            for n in range(N_QC):  # seq chunk of 512
                ps = psum_mm.tile([P, QC], F32, tag="mm")
                for d in range(DM_T):
                    nc.tensor.matmul(
                        ps[:],
                        (w_sb[:, d * COLS + c * P : d * COLS + (c + 1) * P]),
                        (xt[:, d * SEQ + n * QC : d * SEQ + (n + 1) * QC]),
                        start=(d == 0),
                        stop=(d == DM_T - 1),
                    )
                stg = qkstg_pool.tile([P, QC], BF16, tag="qkstg")
                nc.vector.tensor_copy(stg[:], ps[:])
                nc.sync.dma_start(
                    qk_dram_v[:, w_i, c, n * QC : (n + 1) * QC], stg[:]
                )
    qkstg_pool.release()
    wqk_pool.release()

    # ===== phase C: V (natural layout) + ones columns =====================
    oT_pool = tc.alloc_tile_pool(name="oT", bufs=1)
    oT = oT_pool.tile([P, 4 * SEQ], F32R, tag="oT")  # 4 head-pair tiles

    vaug_pool = tc.alloc_tile_pool(name="vaug", bufs=1)
    # V in natural [seq, cols] layout + per-head ones column (softmax denom
    # rides the AV matmul as output row 64): seq-tile-major, 8 x (64 V + 1)
    vaug = vaug_pool.tile([P, SEQ_T * 8 * (HD + 1)], BF16, tag="vaug")
    vaug_v = vaug[:].rearrange("p (s h e) -> p s h e", s=SEQ_T, h=8, e=HD + 1)

    wv_pool = tc.alloc_tile_pool(name="wv", bufs=1)
    wv_sb = wv_pool.tile([P, DM_T * COLS], F32R, tag="wv_sb")
    wstg2_pool = tc.alloc_tile_pool(name="wstg2", bufs=3)
    for d in range(DM_T):
        wst = wstg2_pool.tile([P, COLS], F32, tag="wst2")
        nc.sync.dma_start(wst[:], wv[d * P : (d + 1) * P, :])
        nc.vector.tensor_copy(wv_sb[:, d * COLS : (d + 1) * COLS], wst[:])
    wstg2_pool.release()
    for s in range(SEQ_T):
        ps = psum_mm.tile([P, QC], F32, tag="mm")
        for d in range(DM_T):
            nc.tensor.matmul(
                ps[:],
                (xt[:, d * SEQ + s * P : d * SEQ + (s + 1) * P]),
                (wv_sb[:, d * COLS : (d + 1) * COLS]),
                start=(d == 0),
                stop=(d == DM_T - 1),
            )
        nc.vector.tensor_copy(
            vaug_v[:, s, :, 0:HD],
            ps[:].rearrange("p (h e) -> p h e", h=8),
        )
        nc.vector.tensor_copy(
            vaug_v[:, s, :, HD : HD + 1],
            cstage[:, 0:8].rearrange("p (a b) -> p a b", b=1),
        )
    wv_pool.release()

    # ===== phase D: attention =============================================
    qkpair_pool = tc.alloc_tile_pool(name="qkpair", bufs=2)
    pt_pool = tc.alloc_tile_pool(name="ptile", bufs=4)
    recip_pool = tc.alloc_tile_pool(name="recip", bufs=4)

    for hp in range(4):  # head pair
        # this pair's Q^T and K^T: [128 cols, 2048 seq] each
        qT = qkpair_pool.tile([P, SEQ], BF16, tag="qTp", name=f"qTp_{hp}")
        kT = qkpair_pool.tile([P, SEQ], BF16, tag="kTp", name=f"kTp_{hp}")
        nc.sync.dma_start(qT[:], qk_dram_v[:, 0, hp, :])
        nc.sync.dma_start(kT[:], qk_dram_v[:, 1, hp, :])
        for qc in range(N_QC):
            po = {}
            for hh in range(2):
                po[hh] = psum_acc.tile(
                    [P, QC], F32, tag="po", name=f"po_{hp}_{qc}_{hh}"
                )
            nkt = KT_PER_QC * qc + KT_PER_QC
            for kt in range(nkt):
                for hh in range(2):
                    base = hh * HD
                    ps_s = psum_mm.tile([P, QC], F32, tag="mm")
                    nc.tensor.matmul(
                        ps_s[:],
                        (kT[base : base + HD, kt * P : (kt + 1) * P]),
                        (qT[base : base + HD, qc * QC : (qc + 1) * QC]),
                        start=True,
                        stop=True,
                    )
                    pt = pt_pool.tile([P, QC], BF16, tag="pt")
                    if kt // KT_PER_QC == qc:
                        off = (kt - KT_PER_QC * qc) * P
                        if off > 0:
                            nc.vector.tensor_copy(
                                pt[:, 0:off], zeros384[:, 0:off]
                            )
                        nc.scalar.activation(
                            pt[:, off:QC], ps_s[:, off:QC], AF.Exp, scale=RSQRT
                        )
                        nc.vector.tensor_mul(
                            pt[:, off : off + P], pt[:, off : off + P], mask01[:]
                        )
                    else:
                        nc.scalar.activation(pt[:], ps_s[:], AF.Exp, scale=RSQRT)
                    # AV + softmax denominator (ones column -> output row 64)
                    h = 2 * hp + hh
                    nc.tensor.matmul(
                        po[hh][0 : HD + 1, :],
                        (vaug_v[:, kt, h, :]),
                        (pt[:]),
                        start=(kt == 0),
                        stop=(kt == nkt - 1),
                    )
            for hh in range(2):
                d1 = recip_pool.tile([1, QC], F32, tag="d1")
                nc.vector.tensor_copy(d1[:], po[hh][HD : HD + 1, :])
                d64 = recip_pool.tile([HD, QC], F32, tag="d64")
                nc.gpsimd.partition_broadcast(d64[:], d1[:])
                rc = recip_pool.tile([HD, QC], F32, tag="rc")
                nc.vector.reciprocal(rc[:], d64[:])
                nc.vector.tensor_mul(
                    oT[hh * HD : (hh + 1) * HD,
                       hp * SEQ + qc * QC : hp * SEQ + (qc + 1) * QC],
                    po[hh][0:HD, :],
                    rc[:],
                )
    recip_pool.release()
    pt_pool.release()
    qkpair_pool.release()
    vaug_pool.release()

    # ===== phase E: projection ============================================
    wp_pool = tc.alloc_tile_pool(name="wp", bufs=1)
    wp_sb = wp_pool.tile([P, 4 * DM], F32R, tag="wp_sb")
    wstg3_pool = tc.alloc_tile_pool(name="wstg3", bufs=2)
    for c in range(4):
        wst = wstg3_pool.tile([P, DM], F32, tag="wst3")
        nc.sync.dma_start(wst[:], wp[c * P : (c + 1) * P, :])
        nc.vector.tensor_copy(wp_sb[:, c * DM : (c + 1) * DM], wst[:])
    wstg3_pool.release()
    ostage_pool = tc.alloc_tile_pool(name="ostage", bufs=3)
    for s in range(SEQ_T):
        ost = ostage_pool.tile([P, DM], F32, tag="ost")
        for n2 in range(2):
            ps = psum_mm.tile([P, QC], F32, tag="mm")
            for c in range(4):
                nc.tensor.matmul(
                    ps[:],
                    (oT[:, c * SEQ + s * P : c * SEQ + (s + 1) * P]),
                    (wp_sb[:, c * DM + n2 * QC : c * DM + (n2 + 1) * QC]),
                    start=(c == 0),
                    stop=(c == 3),
                )
            nc.vector.tensor_copy(ost[:, n2 * QC : (n2 + 1) * QC], ps[:])
        nc.sync.dma_start(out[s * P : (s + 1) * P, :], ost[:])
    ostage_pool.release()
    wp_pool.release()
    oT_pool.release()

    qk_dram_pool.release()
    xt_pool.release()
    psum_acc.release()
    psum_mm.release()
    const_pool.release()


_NC_CACHE = None


def _get_program():
    global _NC_CACHE
    if _NC_CACHE is None:
        _NC_CACHE = _build_core_program()
    return _NC_CACHE


def _make_in_maps(x, w_qkv, w_proj):
    x = np.ascontiguousarray(np.asarray(x, dtype=np.float32))
    w_qkv = np.ascontiguousarray(np.asarray(w_qkv, dtype=np.float32))
    w_proj = np.ascontiguousarray(np.asarray(w_proj, dtype=np.float32))
    in_maps = []
    for core in range(N_CORES):
        b, g = core // 2, core % 2
        cs = slice(g * COLS, (g + 1) * COLS)
        in_maps.append(
            {
                "x": np.ascontiguousarray(x[b]),
                "wq": np.ascontiguousarray(w_qkv[:, 0 * DM : 1 * DM][:, cs]),
                "wk": np.ascontiguousarray(w_qkv[:, 1 * DM : 2 * DM][:, cs]),
                "wv": np.ascontiguousarray(w_qkv[:, 2 * DM : 3 * DM][:, cs]),
                "wp": np.ascontiguousarray(w_proj[cs, :]),
            }
        )
    return in_maps


def run_on_hw(x, w_qkv, w_proj, trace=False, **kwargs):
    """Run the SPMD program on 8 cores; returns (full_output, BassKernelResults)."""
    nc = _get_program()
    in_maps = _make_in_maps(x, w_qkv, w_proj)
    res = run_bass_kernel_spmd(
        nc, in_maps, list(range(N_CORES)), trace=trace, **kwargs
    )
    bs = 4
    outp = np.empty((bs, SEQ, DM), dtype=np.float32)
    for b in range(bs):
        outp[b] = res.results[2 * b]["out"] + res.results[2 * b + 1]["out"]
    return outp, res


def kernel(x, w_qkv, w_proj):
    outp, _ = run_on_hw(x, w_qkv, w_proj, trace=False)
    return outp
